# revision 2
# baseline (speedup 1.0000x reference)
"""GRASS encoder kernel for 8 Trainium2 NeuronCores.

Key observations exploited here:

1. The reference returns ``root[0]`` — only batch example 0's root code
   (a [1024] f32 vector) is the output.  Work on examples 1..255 is dead.
2. The stack-machine control flow depends only on ``operations`` (known
   host-side when ``kernel()`` is called), not on tensor data.  We simulate
   the pointer machine symbolically on the host, then backward-slice from
   the root to get the minimal DAG of adj/sym encoder evaluations needed
   (2 nodes for the canonical [1,0,2,3]*K schedule).
3. Fast path (canonical 2-node DAG): both MLP nodes are sharded across all
   8 cores.  Layer 1 is H-sharded (each core computes 256 of the 2048
   hidden units), layer 2 is K-sharded (each core contracts its own hidden
   slice against its 256 rows of W2), giving a per-core partial of the
   2048-dim pre-activation.  The adj partials are combined on-device with
   one AllGather + DVE tree reduction (+ a warm-up collective so the
   cross-core entry barrier and ncfw startup don't sit on the critical
   path); the sym (root) partials are combined on the host.
4. The box encodings (12 -> 1024, two vectors) are computed on the host in
   f32 — they are a few Kflop and would otherwise serialize device work.
5. All device activations are kept in "K-major" layout ([128, n] tiles
   with v[j*128+p] at [p, j]) so they feed matmul contractions directly.
   Layer-1 row outputs are returned to K-major with PE transposes.
6. Weight slices are packed host-side in exactly the SBUF layouts the
   kernel wants (fp16), so every big DMA is a contiguous copy.  Bulk
   weight DMAs ride the sync (SP) HWDGE ring; small latency-critical
   transfers (collective bounce, outputs) ride the scalar (ACT) ring so
   they are never queued behind megabyte weight traffic.
"""

import numpy as np

F, H, BOX, SYMD = 1024, 2048, 12, 8
N_BOX, N_SYM = 32, 16
MAX_STACK, MAX_SYMSTK = 20, 4
NCORES = 8
HC = H // NCORES          # hidden slice per core (256)
MC = HC // 128            # 128-chunks of the hidden slice per core (2)
KJ = F // 128             # contraction 128-chunks of F (8)

_CACHE: dict = {}


# ---------------------------------------------------------------------------
# Host-side symbolic stack simulation + backward slicing (example 0 only)
# ---------------------------------------------------------------------------

def _build_slice(ops0):
    """Return (nodes, root_src) for example 0's op string.

    nodes: list of ('adj', lsrc, rsrc) | ('sym', fsrc, ssrc) in topo order.
    srcs: ('box', i) (tanh(inputStacks[i,0] @ box_W + box_b)),
          ('symvec', j) (symmetryStacks[j,0]), ('node', k), or None (zeros).
    Pointer semantics mirror reference.py exactly: gathers clip to the valid
    range (jnp.take_along_axis), scatters drop when out of bounds (.at.set).
    """
    stack = [None] * MAX_STACK
    symstk = [None] * MAX_SYMSTK
    stack[0] = stack[1] = ('box', 0)
    symstk[0] = symstk[1] = ('symvec', 0)
    sptr, yptr, bptr, qptr = 2, 2, N_BOX - 1, N_SYM - 1
    nodes = []
    clip = lambda v, lo, hi: max(lo, min(hi, v))
    for op in ops0:
        op = int(op)
        pv = ('box', clip(bptr, 0, N_BOX - 1))
        sv = ('symvec', clip(qptr, 0, N_SYM - 1))
        top = stack[clip(sptr - 1, 0, MAX_STACK - 1)]
        sec = stack[clip(sptr - 2, 0, MAX_STACK - 1)]
        stop = symstk[clip(yptr - 1, 0, MAX_SYMSTK - 1)]
        adj = ('node', len(nodes))
        sym = ('node', len(nodes) + 1)
        nodes.append(('adj', sec, top))
        nodes.append(('sym', top, stop))
        push, madj, psym = op <= 1, op == 2, op == 1
        wv = pv if push else (adj if madj else sym)
        wi = sptr if push else (sptr - 2 if madj else sptr - 1)
        if 0 <= wi < MAX_STACK:
            stack[wi] = wv
        if psym:
            symstk[clip(yptr, 0, MAX_SYMSTK - 1)] = sv
        sptr += 1 if push else (-1 if madj else 0)
        yptr += (1 if psym else 0) - (1 if op == 3 else 0)
        bptr -= 1 if push else 0
        qptr -= 1 if psym else 0
    root_src = stack[clip(sptr - 1, 0, MAX_STACK - 1)]

    needed = set()

    def visit(src):
        if src is not None and src[0] == 'node' and src[1] not in needed:
            needed.add(src[1])
            _, a, b = nodes[src[1]]
            visit(a)
            visit(b)

    visit(root_src)
    order = sorted(needed)
    remap = {k: i for i, k in enumerate(order)}
    rn = lambda s: ('node', remap[s[1]]) if (s is not None and s[0] == 'node') else s
    sliced = [(nodes[k][0], rn(nodes[k][1]), rn(nodes[k][2])) for k in order]
    return sliced, rn(root_src)


def _collect_leaves(nodes, root):
    """Ordered unique box / symvec indices referenced by the DAG."""
    boxes, syms, zeros = [], [], False

    def add(src):
        nonlocal zeros
        if src is None:
            zeros = True
        elif src[0] == 'box' and src[1] not in boxes:
            boxes.append(src[1])
        elif src[0] == 'symvec' and src[1] not in syms:
            syms.append(src[1])

    for _, a, b in nodes:
        add(a)
        add(b)
    add(root)
    return boxes, syms, zeros


def _canonical(nodes, root):
    return (len(nodes) == 2 and nodes[0][0] == 'adj'
            and nodes[0][1] is not None and nodes[0][1][0] == 'box'
            and nodes[0][2] is not None and nodes[0][2][0] == 'box'
            and nodes[1][0] == 'sym' and nodes[1][1] == ('node', 0)
            and nodes[1][2] is not None and nodes[1][2][0] == 'symvec'
            and root == ('node', 1))


# ---------------------------------------------------------------------------
# Fast path: canonical DAG, fully sharded, one AllGather
# ---------------------------------------------------------------------------

def _build_program_fast():
    import concourse.bacc as bacc
    import concourse.mybir as mybir
    import concourse.tile as tile

    dt = mybir.dt.float32
    dt16 = mybir.dt.float16
    Tanh = mybir.ActivationFunctionType.Tanh
    nc = bacc.Bacc("TRN2", target_bir_lowering=False, debug=False,
                   enable_asserts=False, num_devices=NCORES)

    def din(name, shape, dty):
        return nc.dram_tensor(name, list(shape), dty, kind="ExternalInput")

    d_ek = din("ek", [128, 16], dt16)          # e3 K-major cols 0-7, e2 8-15
    d_sv9w = din("sv9w", [16, 257], dt16)      # col0: sv9; cols1+: swr9 block
    d_abl = din("abl", [1, 256], dt16)         # adj layer-1 bias row (slice)
    d_b2k = din("b2k", [128, KJ], dt)          # adj_b2 K-major
    d_onesf = din("onesf", [1, 1], dt)         # 1.0 (transpose rhs + warmup)
    d_ones1h = din("ones1h", [1, 1], dt16)     # 1.0 fp16 (bias matmul lhsT)
    d_aw1 = din("aw1", [128, 16 * 256], dt16)  # L1 adj: [awl_j | awr_j] * 8
    d_aw2 = din("aw2", [128, 16 * 128], dt16)  # L2 adj tiles (k,mz)
    d_sw1 = din("sw1", [128, 8 * 256], dt16)   # L1 sym (swl_j) * 8
    d_sw2 = din("sw2", [128, 16 * 128], dt16)  # L2 sym tiles (k,mz)
    d_pout = nc.dram_tensor("pout_k", [128, KJ], dt, kind="ExternalOutput")

    groups = [list(range(NCORES))]

    with tile.TileContext(nc) as tc:
        with (
            tc.tile_pool(name="wp", bufs=1) as wp,
            tc.tile_pool(name="pp", bufs=1, space="PSUM") as pp,
            tc.tile_pool(name="dp", bufs=1, space="DRAM") as dp,
        ):
            def load(dram, shape, tag, dty=dt16, eng=None):
                t = wp.tile(list(shape), dty, tag=tag)
                (eng or nc.scalar).dma_start(t[:], dram[:])
                return t

            # --- small latency-critical loads (ACT HWDGE ring) ---
            t_one1f = load(d_onesf, [1, 1], "one1f", dt)
            t_one1h = load(d_ones1h, [1, 1], "one1h")
            t_ek = load(d_ek, [128, 16], "ek")
            t_sv9w = load(d_sv9w, [16, 257], "sv9w")
            t_abl = load(d_abl, [1, 256], "abl")
            t_b2k = load(d_b2k, [128, KJ], "b2k", dt)

            # --- warm-up collective: absorb the cross-core entry barrier +
            # ncfw startup concurrently with the adj node's compute.
            # Value-neutral: gathers 1.0s and rewrites t_one1f (already 1.0),
            # which also keeps the warm-up live through DCE.
            warm_in = dp.tile([1, 1], dt, tag="warmin")
            warm_out = dp.tile([NCORES, 1], dt, tag="warmout")
            nc.gpsimd.dma_start(warm_in[:], d_onesf[:])
            nc.gpsimd.collective_compute(
                "AllGather", mybir.AluOpType.bypass,
                replica_groups=groups,
                ins=[warm_in[:].opt()], outs=[warm_out[:].opt()])
            nc.gpsimd.dma_start(t_one1f[:], warm_out[0:1, :])

            # --- bulk weights (SP HWDGE ring), in consumption order ---
            t_aw1 = wp.tile([128, 16 * 256], dt16, tag="aw1")
            nc.sync.dma_start(t_aw1[:, 0:2048], d_aw1[:, 0:2048])
            nc.sync.dma_start(t_aw1[:, 2048:4096], d_aw1[:, 2048:4096])
            t_aw2 = wp.tile([128, 16 * 128], dt16, tag="aw2")
            nc.sync.dma_start(t_aw2[:], d_aw2[:])
            t_sw1 = wp.tile([128, 8 * 256], dt16, tag="sw1")
            nc.sync.dma_start(t_sw1[:], d_sw1[:])
            t_sw2 = wp.tile([128, 16 * 128], dt16, tag="sw2")
            nc.sync.dma_start(t_sw2[:], d_sw2[:])

            def layer1(wl_blk, extra, tag):
                """x-stationary GEMV: h_row[1, 256] = sum_j ek_j.T @ W_j."""
                ps1 = pp.tile([1, 256], dt, tag="ps1")
                n_mm = len(wl_blk) + 1
                i = 0
                for lhs_col, w_ap in wl_blk:
                    nc.tensor.matmul(ps1[:, :], lhs_col, w_ap,
                                     start=(i == 0), stop=False)
                    i += 1
                extra(ps1)  # last MM must carry stop=True
                t_row = wp.tile([1, 256], dt, tag=f"row{tag}")
                nc.scalar.activation(t_row[:], ps1[:], Tanh)
                # PE transposes back to K-major [128, MC]
                psT = pp.tile([128, MC], dt, tag="psT")
                for k in range(MC):
                    nc.tensor.matmul(psT[:, k:k + 1],
                                     t_row[0:1, k * 128:(k + 1) * 128],
                                     t_one1f[:, :], is_transpose=True,
                                     start=True, stop=True)
                t_hk = wp.tile([128, MC], dt16, tag=f"hk{tag}")
                nc.vector.tensor_copy(t_hk[:], psT[:])
                return t_hk

            def layer2(t_hk, t_w2, tag):
                """W-stationary: z K-major [128, KJ] f32 in PSUM."""
                psz = pp.tile([128, KJ], dt, tag="psz")
                for mz in range(KJ):
                    for k in range(MC):
                        nc.tensor.matmul(
                            psz[:, mz:mz + 1],
                            t_w2[:, (k * KJ + mz) * 128:(k * KJ + mz + 1) * 128],
                            t_hk[:, k:k + 1],
                            start=(k == 0), stop=(k == MC - 1))
                return psz

            # ---------------- adj node ----------------
            adj_blk = []
            for j in range(KJ):
                adj_blk.append((t_ek[:, j:j + 1],
                                t_aw1[:, j * 512:j * 512 + 256]))
                adj_blk.append((t_ek[:, 8 + j:8 + j + 1],
                                t_aw1[:, j * 512 + 256:j * 512 + 512]))

            def adj_bias(ps1):
                nc.tensor.matmul(ps1[:, :], t_one1h[:, :], t_abl[:, :],
                                 start=False, stop=True)

            t_hk_a = layer1(adj_blk, adj_bias, "a")
            psz_a = layer2(t_hk_a, t_aw2, "a")
            t_zs = wp.tile([128, KJ], dt, tag="zs")
            nc.vector.tensor_copy(t_zs[:], psz_a[:])

            # ---------------- exchange: AllGather z partials ----------------
            ccin = dp.tile([128, KJ], dt, tag="ccin")
            ccout = dp.tile([NCORES, 128, KJ], dt, tag="ccout")
            nc.scalar.dma_start(ccin[:], t_zs[:])
            nc.gpsimd.collective_compute(
                "AllGather", mybir.AluOpType.bypass,
                replica_groups=groups,
                ins=[ccin[:].opt()], outs=[ccout[:].opt()])
            t_P = wp.tile([128, NCORES * KJ], dt, tag="P")
            for r in range(NCORES):
                nc.scalar.dma_start(t_P[:, r * KJ:(r + 1) * KJ], ccout[r])

            # tree-reduce the 8 rank blocks, add bias, tanh -> adj K-major
            nc.vector.tensor_add(t_P[:, 0:32], t_P[:, 0:32], t_P[:, 32:64])
            nc.vector.tensor_add(t_P[:, 0:16], t_P[:, 0:16], t_P[:, 16:32])
            nc.vector.tensor_add(t_P[:, 0:8], t_P[:, 0:8], t_P[:, 8:16])
            nc.vector.tensor_add(t_P[:, 0:8], t_P[:, 0:8], t_b2k[:])
            t_adjk = wp.tile([128, KJ], dt16, tag="adjk")
            nc.scalar.activation(t_adjk[:], t_P[:, 0:8], Tanh)

            # ---------------- sym node ----------------
            sym_blk = [(t_adjk[:, j:j + 1], t_sw1[:, j * 256:(j + 1) * 256])
                       for j in range(KJ)]

            def sym_extra(ps1):
                # sv @ sym_Wr + (sym_bl + sym_br), K=16 (row 8 is the bias
                # row with sv9[8] = 1; rows 9-15 are zero padding)
                nc.tensor.matmul(ps1[:, :], t_sv9w[:, 0:1], t_sv9w[:, 1:257],
                                 start=False, stop=True)

            t_hk_s = layer1(sym_blk, sym_extra, "s")
            psz_s = layer2(t_hk_s, t_sw2, "s")
            t_os = wp.tile([128, KJ], dt, tag="os")
            nc.vector.tensor_copy(t_os[:], psz_s[:])
            nc.scalar.dma_start(d_pout[:], t_os[:])

    nc.compile()
    return nc


def _pack_inputs_fast(inputs):
    f32, f16 = np.float32, np.float16
    g = lambda k: np.asarray(inputs[k], dtype=f32)
    nodes_meta = inputs['_meta']  # (i3, i2, jsym) leaf indices
    i3, i2, jsym = nodes_meta

    x3 = g('inputStacks')[i3, 0]
    x2 = g('inputStacks')[i2, 0]
    sv = g('symmetryStacks')[jsym, 0]
    e3 = np.tanh(x3 @ g('box_W') + g('box_b'))
    e2 = np.tanh(x2 @ g('box_W') + g('box_b'))
    ek = np.concatenate([e3.reshape(KJ, 128).T, e2.reshape(KJ, 128).T],
                        axis=1).astype(f16)           # [128, 16]

    adj_Wl, adj_Wr, adj_W2 = g('adj_Wl'), g('adj_Wr'), g('adj_W2')
    sym_Wl, sym_W2, sym_Wr = g('sym_Wl'), g('sym_W2'), g('sym_Wr')
    sym_b1 = g('sym_bl') + g('sym_br')
    adj_bl = g('adj_bl')
    b2k = np.ascontiguousarray(g('adj_b2').reshape(KJ, 128).T)  # [128, 8] f32
    onesf = np.ones((1, 1), f32)
    ones1h = np.ones((1, 1), f16)

    def pack_w1_pair(Wl, Wr, c):
        # [F, H] x2 -> [128, 8*2*256]: block j = [Wl_j | Wr_j]
        sl = Wl[:, c * HC:(c + 1) * HC].reshape(KJ, 128, HC)
        sr = Wr[:, c * HC:(c + 1) * HC].reshape(KJ, 128, HC)
        out = np.stack([sl, sr], axis=1)              # [8, 2, 128, 256]
        return np.ascontiguousarray(
            out.transpose(2, 0, 1, 3).reshape(128, 16 * 256)).astype(f16)

    def pack_w1(W, c):
        s = W[:, c * HC:(c + 1) * HC].reshape(KJ, 128, HC)
        return np.ascontiguousarray(
            s.transpose(1, 0, 2).reshape(128, KJ * HC)).astype(f16)

    def pack_w2(W, c):
        # rows slice [HC, F] -> W-stationary tiles (k, mz) at col (k*8+mz)*128
        s = W[c * HC:(c + 1) * HC, :].reshape(MC, 128, KJ, 128)
        return np.ascontiguousarray(
            s.transpose(1, 0, 2, 3).reshape(128, MC * KJ * 128)).astype(f16)

    in_maps = []
    for c in range(NCORES):
        sv9w = np.zeros((16, 257), f16)
        sv9w[:SYMD, 0] = sv.astype(f16)
        sv9w[SYMD, 0] = 1.0
        sv9w[:SYMD, 1:] = sym_Wr[:, c * HC:(c + 1) * HC].astype(f16)
        sv9w[SYMD, 1:] = sym_b1[c * HC:(c + 1) * HC].astype(f16)
        in_maps.append({
            "ek": ek,
            "sv9w": sv9w,
            "abl": np.ascontiguousarray(
                adj_bl[None, c * HC:(c + 1) * HC]).astype(f16),
            "b2k": b2k,
            "onesf": onesf,
            "ones1h": ones1h,
            "aw1": pack_w1_pair(adj_Wl, adj_Wr, c),
            "aw2": pack_w2(adj_W2, c),
            "sw1": pack_w1(sym_Wl, c),
            "sw2": pack_w2(sym_W2, c),
        })
    return in_maps


# ---------------------------------------------------------------------------
# General fallback: arbitrary sliced DAGs (H-split + AllGather per node)
# ---------------------------------------------------------------------------

def _build_program(nodes, root, box_pos, sym_pos, nb, ns, need_zero):
    import concourse.bacc as bacc
    import concourse.mybir as mybir
    import concourse.tile as tile

    dt = mybir.dt.float32
    dt16 = mybir.dt.float16
    Tanh = mybir.ActivationFunctionType.Tanh
    nc = bacc.Bacc("TRN2", target_bir_lowering=False, debug=False,
                   enable_asserts=False, num_devices=NCORES)

    def din(name, shape, dty):
        return nc.dram_tensor(name, list(shape), dty, kind="ExternalInput")
    d_xz = din("xz", [BOX + 1, nb], dt16)
    d_boxw = din("boxw", [BOX + 1, F], dt16)
    d_awl = din("awl", [128, KJ * HC], dt16)
    d_awr = din("awr", [128, KJ * HC], dt16)
    d_abl = din("abl", [1, HC], dt16)
    d_aw2 = din("aw2", [128, MC * F], dt16)
    d_ab2 = din("ab2", [1, F], dt)
    d_swl = din("swl", [128, KJ * HC], dt16)
    d_swr9 = din("swr9", [SYMD + 1, HC], dt16)
    d_sw2 = din("sw2", [128, MC * F], dt16)
    d_sb2 = din("sb2", [1, F], dt)
    d_sv1 = din("sv1", [SYMD + 1, ns], dt16)
    d_ones = din("ones9", [NCORES + 1, 1], dt)
    d_ones1h = din("ones1h", [1, 1], dt16)
    d_out = nc.dram_tensor("root_t", [128, KJ], dt, kind="ExternalOutput")
    d_pout = nc.dram_tensor("part_out", [1, F], dt, kind="ExternalOutput")
    host_root = root is not None and root[0] == "node"

    n_adj = sum(1 for t, _, _ in nodes if t == 'adj')
    n_sym = len(nodes) - n_adj
    any_exchange = any(
        not (host_root and k == root[1]) for k in range(len(nodes)))
    groups = [list(range(NCORES))]

    with tile.TileContext(nc) as tc:
        with (
            tc.tile_pool(name="wp", bufs=1) as wp,
            tc.tile_pool(name="sp", bufs=2) as sp,
            tc.tile_pool(name="rp", bufs=1) as rp,
            tc.tile_pool(name="pp", bufs=1, space="PSUM") as pp,
            tc.tile_pool(name="dp", bufs=1, space="DRAM") as dp,
        ):
            def load(dram, shape, tag, dty=dt16):
                t = wp.tile(list(shape), dty, tag=tag)
                nc.sync.dma_start(t[:], dram[:])
                return t

            t_ones = load(d_ones, [NCORES + 1, 1], "ones", dt)
            t_ones1h = load(d_ones1h, [1, 1], "ones1h")
            t_boxw = load(d_boxw, [BOX + 1, F], "boxw")
            t_xz = load(d_xz, [BOX + 1, nb], "xz")
            t_awl = t_awr = t_abl = t_aw2 = None
            t_swl = t_swr9 = t_sw2 = t_sv1 = None
            if n_adj:
                t_awl = load(d_awl, [128, KJ * HC], "awl")
                t_awr = load(d_awr, [128, KJ * HC], "awr")
                t_abl = load(d_abl, [1, HC], "abl")
                t_aw2 = load(d_aw2, [128, MC * F], "aw2")
            if n_sym:
                t_swl = load(d_swl, [128, KJ * HC], "swl")
                t_swr9 = load(d_swr9, [SYMD + 1, HC], "swr9")
                t_sw2 = load(d_sw2, [128, MC * F], "sw2")
                t_sv1 = load(d_sv1, [SYMD + 1, ns], "sv1")
            t_zero = None
            if need_zero:
                t_zero = rp.tile([128, KJ], dt, tag="zero")
                nc.gpsimd.memset(t_zero[:], 0.0)

            if any_exchange:
                warm_in = dp.tile([1, 1], dt, tag="warmin")
                warm_out = dp.tile([NCORES, 1], dt, tag="warmout")
                nc.gpsimd.dma_start(warm_in[:], d_ones[0:1, :])
                nc.gpsimd.collective_compute(
                    "AllGather", mybir.AluOpType.bypass,
                    replica_groups=groups,
                    ins=[warm_in[:].opt()], outs=[warm_out[:].opt()])
                nc.gpsimd.dma_start(t_ones[0:1, :], warm_out[0:1, :])

            # --- box encodings, K-major: col m*nb + t = chunk m of box t ---
            ps_box = pp.tile([128, KJ * nb], dt, tag="psbox")
            for m in range(KJ):
                nc.tensor.matmul(ps_box[:, m * nb:(m + 1) * nb],
                                 t_boxw[:, m * 128:(m + 1) * 128],
                                 t_xz[:], start=True, stop=True)
            t_bx = rp.tile([128, KJ * nb], dt16, tag="bx")
            nc.scalar.activation(t_bx[:], ps_box[:], Tanh)

            res_tiles = []

            def col(src, j):
                """K-major chunk j ([128,1] rhs) of a node-input vector."""
                if src is None:
                    return t_zero[:, j:j + 1]
                if src[0] == 'box':
                    t = box_pos[src[1]]
                    return t_bx[:, j * nb + t:j * nb + t + 1]
                return res_tiles[src[1]][:, j:j + 1]

            for k, (typ, a, b) in enumerate(nodes):
                # ---- layer 1: pre[HC] in K-major [128, MC] ----
                ps1 = pp.tile([128, MC], dt, tag="ps1")
                wl = t_awl if typ == 'adj' else t_swl
                for m in range(MC):
                    for j in range(KJ):
                        nc.tensor.matmul(
                            ps1[:, m:m + 1],
                            wl[:, (j * MC + m) * 128:(j * MC + m + 1) * 128],
                            col(a, j), start=(j == 0), stop=False)
                    if typ == 'adj':
                        for j in range(KJ):
                            nc.tensor.matmul(
                                ps1[:, m:m + 1],
                                t_awr[:, (j * MC + m) * 128:(j * MC + m + 1) * 128],
                                col(b, j), start=False, stop=False)
                        nc.tensor.matmul(ps1[:, m:m + 1],
                                         t_abl[:, m * 128:(m + 1) * 128],
                                         t_ones1h[:, :], start=False, stop=True)
                    else:
                        if b is None:
                            # missing sym param == zeros: keep only the bias row
                            nc.tensor.matmul(ps1[:, m:m + 1],
                                             t_swr9[SYMD:SYMD + 1,
                                                    m * 128:(m + 1) * 128],
                                             t_ones1h[:, :],
                                             start=False, stop=True)
                        else:
                            sc = sym_pos[b[1]]
                            nc.tensor.matmul(ps1[:, m:m + 1],
                                             t_swr9[:, m * 128:(m + 1) * 128],
                                             t_sv1[:, sc:sc + 1],
                                             start=False, stop=True)
                th = sp.tile([128, MC], dt16, tag="h1")
                nc.scalar.activation(th[:], ps1[:], Tanh)

                # ---- layer 2: partial [1, F] (row-major, pre-activation) ----
                w2 = t_aw2 if typ == 'adj' else t_sw2
                ps2a = pp.tile([1, 512], dt, tag="ps2a")
                ps2b = pp.tile([1, 512], dt, tag="ps2b")
                for half, pst in ((0, ps2a), (1, ps2b)):
                    for kk in range(MC):
                        nc.tensor.matmul(
                            pst[:, :],
                            th[:, kk:kk + 1],
                            w2[:, kk * F + half * 512: kk * F + half * 512 + 512],
                            start=(kk == 0), stop=(kk == MC - 1))
                t_part = sp.tile([1, F], dt, tag="part")
                nc.vector.tensor_copy(t_part[0:1, 0:512], ps2a[:, :])
                nc.vector.tensor_copy(t_part[0:1, 512:1024], ps2b[:, :])

                if host_root and k == root[1]:
                    # root node: emit per-core partials; host sums+bias+tanh
                    nc.sync.dma_start(d_pout[:], t_part[:])
                    res_tiles.append(None)
                    continue

                # ---- exchange: AllGather partials, reduce + bias + tanh ----
                ccin = dp.tile([1, F], dt, tag=f"ccin{k}")
                ccout = dp.tile([NCORES, F], dt, tag=f"ccout{k}")
                nc.sync.dma_start(ccin[:], t_part[:])
                nc.gpsimd.collective_compute(
                    "AllGather", mybir.AluOpType.bypass,
                    replica_groups=groups,
                    ins=[ccin[:].opt()], outs=[ccout[:].opt()])
                t_P = sp.tile([NCORES + 1, F], dt, tag="P")
                nc.sync.dma_start(t_P[0:NCORES, :], ccout[:])
                nc.sync.dma_start(t_P[NCORES:NCORES + 1, :],
                                  (d_ab2 if typ == 'adj' else d_sb2)[:])
                psr = pp.tile([128, KJ], dt, tag="psr")
                for m in range(KJ):
                    nc.tensor.matmul(psr[:, m:m + 1],
                                     t_P[:, m * 128:(m + 1) * 128],
                                     t_ones[:, :], start=True, stop=True)
                t_res = rp.tile([128, KJ], dt16, tag=f"res{k}")
                nc.scalar.activation(t_res[:], psr[:], Tanh)
                res_tiles.append(t_res)

            # ---- root -> output ----
            if root is None:
                nc.sync.dma_start(d_out[:], t_zero[:])
            elif root[0] == 'node':
                pass  # root node handled above via part_out
            else:  # box leaf
                t_stage = rp.tile([128, KJ], dt, tag="rootstage")
                t = box_pos[root[1]]
                for j in range(KJ):
                    nc.vector.tensor_copy(t_stage[:, j:j + 1],
                                          t_bx[:, j * nb + t:j * nb + t + 1])
                nc.sync.dma_start(d_out[:], t_stage[:])

    nc.compile()
    return nc


def _pack_inputs(inputs, boxes, syms, nb, ns):
    f32, f16 = np.float32, np.float16
    g = lambda k: np.asarray(inputs[k], dtype=f32)
    inputStacks, symmetryStacks = g('inputStacks'), g('symmetryStacks')

    xz = np.zeros((BOX + 1, nb), f16)
    for t, i in enumerate(boxes):
        xz[:BOX, t] = inputStacks[i, 0].astype(f16)
        xz[BOX, t] = 1.0
    boxw = np.ascontiguousarray(
        np.concatenate([g('box_W'), g('box_b')[None, :]], axis=0)).astype(f16)
    sv1 = np.zeros((SYMD + 1, ns), f16)
    for t, j in enumerate(syms):
        sv1[:SYMD, t] = symmetryStacks[j, 0].astype(f16)
        sv1[SYMD, t] = 1.0
    ones9 = np.ones((NCORES + 1, 1), f32)
    ones1h = np.ones((1, 1), f16)
    ab2 = np.ascontiguousarray(g('adj_b2')[None, :])
    sb2 = np.ascontiguousarray(g('sym_b2')[None, :])

    def pack_w1(W, c):
        s = W[:, c * HC:(c + 1) * HC]
        return np.ascontiguousarray(
            s.reshape(KJ, 128, HC).transpose(1, 0, 2).reshape(
                128, KJ * HC)).astype(f16)

    def pack_w2(W, c):
        s = W[c * HC:(c + 1) * HC, :]
        return np.ascontiguousarray(
            s.reshape(MC, 128, F).transpose(1, 0, 2).reshape(
                128, MC * F)).astype(f16)

    adj_Wl, adj_Wr, adj_W2 = g('adj_Wl'), g('adj_Wr'), g('adj_W2')
    sym_Wl, sym_W2, sym_Wr = g('sym_Wl'), g('sym_W2'), g('sym_Wr')
    sym_b1 = g('sym_bl') + g('sym_br')
    adj_bl = g('adj_bl')

    in_maps = []
    for c in range(NCORES):
        swr9 = np.ascontiguousarray(np.concatenate(
            [sym_Wr[:, c * HC:(c + 1) * HC],
             sym_b1[None, c * HC:(c + 1) * HC]], axis=0)).astype(f16)
        in_maps.append({
            "xz": xz, "boxw": boxw, "sv1": sv1,
            "ones9": ones9, "ones1h": ones1h, "ab2": ab2, "sb2": sb2,
            "awl": pack_w1(adj_Wl, c), "awr": pack_w1(adj_Wr, c),
            "abl": np.ascontiguousarray(
                adj_bl[None, c * HC:(c + 1) * HC]).astype(f16),
            "aw2": pack_w2(adj_W2, c),
            "swl": pack_w1(sym_Wl, c), "swr9": swr9,
            "sw2": pack_w2(sym_W2, c),
        })
    return in_maps


# ---------------------------------------------------------------------------
# Entry point
# ---------------------------------------------------------------------------

def build_for_inputs(inputs):
    """Build (or fetch cached) compiled program + packed inputs."""
    ops = np.asarray(inputs['operations'])
    ops0 = ops[:, 0].astype(np.int64)
    nodes, root = _build_slice(ops0)
    boxes, syms, need_zero = _collect_leaves(nodes, root)
    nb, ns = max(1, len(boxes)), max(1, len(syms))

    use_fast = _canonical(nodes, root)
    key = repr((nodes, root, nb, ns, need_zero, use_fast))
    if key not in _CACHE:
        if use_fast:
            _CACHE[key] = _build_program_fast()
        else:
            box_pos = {b: i for i, b in enumerate(boxes)}
            sym_pos = {s: i for i, s in enumerate(syms)}
            _CACHE[key] = _build_program(nodes, root, box_pos, sym_pos,
                                         nb, ns, need_zero)
    nc = _CACHE[key]
    if use_fast:
        meta = (nodes[0][1][1], nodes[0][2][1], nodes[1][2][1])
        in_maps = _pack_inputs_fast({**inputs, '_meta': meta})
    else:
        box_pos = {b: i for i, b in enumerate(boxes)}
        sym_pos = {s: i for i, s in enumerate(syms)}
        in_maps = _pack_inputs(inputs, boxes, syms, nb, ns)
    return nc, in_maps, (nodes, root, use_fast)


def assemble_output(results, nodes, root, use_fast, inputs):
    """Host-side unshard: combine per-core outputs into the root vector."""
    if use_fast:
        parts = np.zeros((128, KJ), np.float64)
        for c in range(NCORES):
            parts += np.asarray(results[c]["pout_k"], np.float64)
        b2 = np.asarray(inputs['sym_b2'], np.float64).reshape(KJ, 128).T
        return np.tanh(parts + b2).astype(np.float32).T.ravel()
    if root is not None and root[0] == 'node':
        parts = np.stack([np.asarray(results[c]["part_out"], np.float32)[0]
                          for c in range(NCORES)])
        b2 = np.asarray(
            inputs['adj_b2' if nodes[root[1]][0] == 'adj' else 'sym_b2'],
            np.float32)
        return np.tanh(parts.sum(axis=0) + b2).astype(np.float32)
    root_t = np.asarray(results[0]["root_t"], np.float32)
    return np.ascontiguousarray(root_t.T.ravel())


def kernel(**inputs) -> np.ndarray:
    from concourse.bass_utils import run_bass_kernel_spmd

    nc, in_maps, (nodes, root, use_fast) = build_for_inputs(inputs)
    res = run_bass_kernel_spmd(nc, in_maps, core_ids=list(range(NCORES)))
    return assemble_output(res.results, nodes, root, use_fast, inputs)


# revision 6
# speedup vs baseline: 1.6990x; 1.6990x over previous
"""GRASS encoder kernel for 8 Trainium2 NeuronCores.

Key observations exploited here:

1. The reference returns ``root[0]`` — only batch example 0's root code
   (a [1024] f32 vector) is the output.  Work on examples 1..255 is dead.
2. The stack-machine control flow depends only on ``operations`` (known
   host-side when ``kernel()`` is called), not on tensor data.  We simulate
   the pointer machine symbolically on the host, then backward-slice from
   the root to get the minimal DAG of adj/sym encoder evaluations needed
   (2 nodes for the canonical [1,0,2,3]*K schedule).
3. ncfw collectives cost ~45-60us of per-execution framework overhead in
   this environment (measured), so the kernel uses NO collectives: every
   core computes the full adj node (replicated) and its own H-slice of the
   sym (root) node; the host sums the 8 sym partials (+bias, tanh).
4. The execution is DMA-bound (~12.6 MB/core of replicated fp16 adj
   weights).  Two measured facts shape the DMA strategy: (a) concurrent
   HWDGE transfers on one ring round-robin at packet granularity, so a
   transfer issued first can still complete last — defeating pipelining;
   (b) DMA bandwidth is ~300-340 GB/s/core.  We therefore split the bulk
   weights into ~1MB pieces and chain them depth-2 with explicit Tile deps
   so at most 2 are in flight: near-full bandwidth AND in-order completion,
   letting the PE trail the stream closely.  Sym-node weights are chained
   before the adj layer-2 weights since the PE needs them right after z.
5. The box encodings (12 -> 1024, two tanh GEMVs) are computed on the host
   in f32; biases ride the matmul stream (ones-column trick) so no DVE
   bias adds are needed.
"""

import numpy as np

F, H, BOX, SYMD = 1024, 2048, 12, 8
N_BOX, N_SYM = 32, 16
MAX_STACK, MAX_SYMSTK = 20, 4
NCORES = 8
HC = H // NCORES          # hidden slice per core (256)
MC = HC // 128            # 128-chunks of the hidden slice per core (2)
KJ = F // 128             # contraction 128-chunks of F (8)
HK = H // 128             # contraction 128-chunks of H (16)

_CACHE: dict = {}


# ---------------------------------------------------------------------------
# Host-side symbolic stack simulation + backward slicing (example 0 only)
# ---------------------------------------------------------------------------

def _build_slice(ops0):
    """Return (nodes, root_src) for example 0's op string.

    nodes: list of ('adj', lsrc, rsrc) | ('sym', fsrc, ssrc) in topo order.
    srcs: ('box', i) (tanh(inputStacks[i,0] @ box_W + box_b)),
          ('symvec', j) (symmetryStacks[j,0]), ('node', k), or None (zeros).
    Pointer semantics mirror reference.py exactly: gathers clip to the valid
    range (jnp.take_along_axis), scatters drop when out of bounds (.at.set).
    """
    stack = [None] * MAX_STACK
    symstk = [None] * MAX_SYMSTK
    stack[0] = stack[1] = ('box', 0)
    symstk[0] = symstk[1] = ('symvec', 0)
    sptr, yptr, bptr, qptr = 2, 2, N_BOX - 1, N_SYM - 1
    nodes = []
    clip = lambda v, lo, hi: max(lo, min(hi, v))
    for op in ops0:
        op = int(op)
        pv = ('box', clip(bptr, 0, N_BOX - 1))
        sv = ('symvec', clip(qptr, 0, N_SYM - 1))
        top = stack[clip(sptr - 1, 0, MAX_STACK - 1)]
        sec = stack[clip(sptr - 2, 0, MAX_STACK - 1)]
        stop = symstk[clip(yptr - 1, 0, MAX_SYMSTK - 1)]
        adj = ('node', len(nodes))
        sym = ('node', len(nodes) + 1)
        nodes.append(('adj', sec, top))
        nodes.append(('sym', top, stop))
        push, madj, psym = op <= 1, op == 2, op == 1
        wv = pv if push else (adj if madj else sym)
        wi = sptr if push else (sptr - 2 if madj else sptr - 1)
        if 0 <= wi < MAX_STACK:
            stack[wi] = wv
        if psym:
            symstk[clip(yptr, 0, MAX_SYMSTK - 1)] = sv
        sptr += 1 if push else (-1 if madj else 0)
        yptr += (1 if psym else 0) - (1 if op == 3 else 0)
        bptr -= 1 if push else 0
        qptr -= 1 if psym else 0
    root_src = stack[clip(sptr - 1, 0, MAX_STACK - 1)]

    needed = set()

    def visit(src):
        if src is not None and src[0] == 'node' and src[1] not in needed:
            needed.add(src[1])
            _, a, b = nodes[src[1]]
            visit(a)
            visit(b)

    visit(root_src)
    order = sorted(needed)
    remap = {k: i for i, k in enumerate(order)}
    rn = lambda s: ('node', remap[s[1]]) if (s is not None and s[0] == 'node') else s
    sliced = [(nodes[k][0], rn(nodes[k][1]), rn(nodes[k][2])) for k in order]
    return sliced, rn(root_src)


def _collect_leaves(nodes, root):
    """Ordered unique box / symvec indices referenced by the DAG."""
    boxes, syms, zeros = [], [], False

    def add(src):
        nonlocal zeros
        if src is None:
            zeros = True
        elif src[0] == 'box' and src[1] not in boxes:
            boxes.append(src[1])
        elif src[0] == 'symvec' and src[1] not in syms:
            syms.append(src[1])

    for _, a, b in nodes:
        add(a)
        add(b)
    add(root)
    return boxes, syms, zeros


def _canonical(nodes, root):
    return (len(nodes) == 2 and nodes[0][0] == 'adj'
            and nodes[0][1] is not None and nodes[0][1][0] == 'box'
            and nodes[0][2] is not None and nodes[0][2][0] == 'box'
            and nodes[1][0] == 'sym' and nodes[1][1] == ('node', 0)
            and nodes[1][2] is not None and nodes[1][2][0] == 'symvec'
            and root == ('node', 1))


# ---------------------------------------------------------------------------
# Fast path: canonical DAG, zero collectives, chained DMA stream
# ---------------------------------------------------------------------------

def _build_program_fast():
    import concourse.bacc as bacc
    import concourse.mybir as mybir
    import concourse.tile as tile
    from concourse.tile import add_dep_helper

    dt = mybir.dt.float32
    dt16 = mybir.dt.float16
    Tanh = mybir.ActivationFunctionType.Tanh
    nc = bacc.Bacc("TRN2", target_bir_lowering=False, debug=False,
                   enable_asserts=False, num_devices=NCORES)

    def din(name, shape, dty):
        return nc.dram_tensor(name, list(shape), dty, kind="ExternalInput")

    d_ek = din("ek", [128, 16], dt16)            # e3 K-major 0-7, e2 8-15
    d_ablf = din("ablf", [1, H], dt16)           # adj layer-1 bias row
    d_b2row = din("b2row", [1, F], dt16)         # adj_b2 row
    d_sv9w = din("sv9w", [16, 1 + HC], dt16)     # col0 sv9; cols1+: swr9
    d_onesf = din("onesf", [1, 1], dt)
    d_ones1h = din("ones1h", [1, 1], dt16)
    d_awlf = din("awlf", [128, KJ * H], dt16)    # Wl full, row-pack
    d_awrf = din("awrf", [128, KJ * H], dt16)    # Wr full, row-pack
    d_aw2f = din("aw2f", [128, HK * F], dt16)    # W2 full, row-pack
    d_swl = din("swl", [128, KJ * HC], dt16)     # sym_Wl slice, K-major pack
    d_sw2t = din("sw2t", [128, MC * KJ * 128], dt16)  # sym_W2 slice tiles
    d_pout = nc.dram_tensor("pout_k", [128, KJ], dt, kind="ExternalOutput")

    with tile.TileContext(nc) as tc:
        with (
            tc.tile_pool(name="wp", bufs=1) as wp,
            tc.tile_pool(name="pp", bufs=1, space="PSUM") as pp,
        ):
            # --- small loads on the scalar (ACT) HWDGE ring ---
            def sload(dram, shape, tag, dty=dt16):
                t = wp.tile(list(shape), dty, tag=tag)
                nc.scalar.dma_start(t[:], dram[:])
                return t

            t_ones1f = sload(d_onesf, [1, 1], "ones1f", dt)
            t_ones1h = sload(d_ones1h, [1, 1], "ones1h")
            t_ek = sload(d_ek, [128, 16], "ek")
            t_ablf = sload(d_ablf, [1, H], "ablf")
            t_b2row = sload(d_b2row, [1, F], "b2row")
            t_sv9w = sload(d_sv9w, [16, 1 + HC], "sv9w")

            # --- bulk weights: chained ~1MB pieces on the sync ring.
            # Depth-2 chaining keeps at most 2 transfers in flight, so
            # packet round-robin cannot starve the piece the PE needs next.
            t_awlf = wp.tile([128, KJ * H], dt16, tag="awlf")
            t_awrf = wp.tile([128, KJ * H], dt16, tag="awrf")
            t_aw2f = wp.tile([128, HK * F], dt16, tag="aw2f")
            t_swl = wp.tile([128, KJ * HC], dt16, tag="swl")
            t_sw2t = wp.tile([128, MC * KJ * 128], dt16, tag="sw2t")

            pieces = []
            for p in range(4):       # Wl: 2 j-blocks per piece (1MB)
                sl = slice(p * 2 * H, (p + 1) * 2 * H)
                pieces.append((t_awlf, d_awlf, sl))
            for p in range(4):       # Wr
                sl = slice(p * 2 * H, (p + 1) * 2 * H)
                pieces.append((t_awrf, d_awrf, sl))
            # sym weights before aw2f: the PE needs them right after z
            pieces.append((t_swl, d_swl, slice(0, KJ * HC)))
            pieces.append((t_sw2t, d_sw2t, slice(0, MC * KJ * 128)))
            for p in range(4):       # W2: 4 k-chunks per piece (1MB)
                sl = slice(p * 4 * F, (p + 1) * 4 * F)
                pieces.append((t_aw2f, d_aw2f, sl))

            chain = []
            for t, dsrc, sl in pieces:
                inst = nc.sync.dma_start(t[:, sl], dsrc[:, sl])
                if len(chain) >= 2:
                    add_dep_helper(inst.ins, chain[-2].ins, sync=True,
                                   reason="bulk DMA depth-2 chain")
                chain.append(inst)

            # ---- adj layer 1: x-stationary, h row [1, H] in 4 psum banks
            ps_row = [pp.tile([1, 512], dt, tag=f"ph{n}", name=f"ph{n}")
                      for n in range(4)]
            for j in range(KJ):
                for n in range(4):
                    nc.tensor.matmul(
                        ps_row[n][:, :], t_ek[:, j:j + 1],
                        t_awlf[:, j * H + n * 512:j * H + (n + 1) * 512],
                        start=(j == 0), stop=False)
            for j in range(KJ):
                for n in range(4):
                    nc.tensor.matmul(
                        ps_row[n][:, :], t_ek[:, 8 + j:8 + j + 1],
                        t_awrf[:, j * H + n * 512:j * H + (n + 1) * 512],
                        start=False, stop=False)
            for n in range(4):
                nc.tensor.matmul(ps_row[n][:, :], t_ones1h[:, :],
                                 t_ablf[:, n * 512:(n + 1) * 512],
                                 start=False, stop=True)
            t_h1row = wp.tile([1, H], dt, tag="h1row")
            for n in range(4):
                nc.scalar.activation(t_h1row[0:1, n * 512:(n + 1) * 512],
                                     ps_row[n][:, :], Tanh)

            # transpose h1 row -> K-major [128, HK] fp16
            ps_tr = pp.tile([128, HK], dt, tag="pstr")
            for c in range(HK):
                nc.tensor.matmul(ps_tr[:, c:c + 1],
                                 t_h1row[0:1, c * 128:(c + 1) * 128],
                                 t_ones1f[:, :], is_transpose=True,
                                 start=True, stop=True)
            t_h1t = wp.tile([128, HK], dt16, tag="h1t")
            nc.vector.tensor_copy(t_h1t[:], ps_tr[:])

            # ---- adj layer 2: x-stationary, z row [1, F] (+bias in-psum)
            ps2 = [pp.tile([1, 512], dt, tag=f"pz{n}", name=f"pz{n}")
                   for n in range(2)]
            for k in range(HK):
                for half in range(2):
                    nc.tensor.matmul(
                        ps2[half][:, :], t_h1t[:, k:k + 1],
                        t_aw2f[:, k * F + half * 512:k * F + half * 512 + 512],
                        start=(k == 0), stop=False)
            for half in range(2):
                nc.tensor.matmul(ps2[half][:, :], t_ones1h[:, :],
                                 t_b2row[:, half * 512:half * 512 + 512],
                                 start=False, stop=True)
            t_adjrow = wp.tile([1, F], dt, tag="adjrow")
            for half in range(2):
                nc.scalar.activation(t_adjrow[0:1, half * 512:half * 512 + 512],
                                     ps2[half][:, :], Tanh)

            # transpose adj row -> K-major [128, KJ] fp16
            ps_adj = pp.tile([128, KJ], dt, tag="pstr")
            for c in range(KJ):
                nc.tensor.matmul(ps_adj[:, c:c + 1],
                                 t_adjrow[0:1, c * 128:(c + 1) * 128],
                                 t_ones1f[:, :], is_transpose=True,
                                 start=True, stop=True)
            t_adjt = wp.tile([128, KJ], dt16, tag="adjt")
            nc.vector.tensor_copy(t_adjt[:], ps_adj[:])

            # ---- sym node, H-sliced ----
            # layer 1: W-stationary K-major [128, MC]; bias rides sv9w row 8
            ps1 = pp.tile([128, MC], dt, tag="ph0")
            for m in range(MC):
                for j in range(KJ):
                    nc.tensor.matmul(
                        ps1[:, m:m + 1],
                        t_swl[:, (j * MC + m) * 128:(j * MC + m + 1) * 128],
                        t_adjt[:, j:j + 1], start=(j == 0), stop=False)
                nc.tensor.matmul(ps1[:, m:m + 1],
                                 t_sv9w[:, 1 + m * 128:1 + (m + 1) * 128],
                                 t_sv9w[:, 0:1], start=False, stop=True)
            th = wp.tile([128, MC], dt16, tag="th")
            nc.scalar.activation(th[:], ps1[:], Tanh)

            # layer 2: W-stationary tiles -> partial K-major [128, KJ] f32
            psz = pp.tile([128, KJ], dt, tag="pz0")
            for mz in range(KJ):
                for k in range(MC):
                    nc.tensor.matmul(
                        psz[:, mz:mz + 1],
                        t_sw2t[:, (k * KJ + mz) * 128:(k * KJ + mz + 1) * 128],
                        th[:, k:k + 1], start=(k == 0), stop=(k == MC - 1))
            t_os = wp.tile([128, KJ], dt, tag="os")
            nc.vector.tensor_copy(t_os[:], psz[:])
            nc.scalar.dma_start(d_pout[:], t_os[:])

    nc.compile()
    return nc


def _pack_inputs_fast(inputs, meta):
    f32, f16 = np.float32, np.float16
    g = lambda k: np.asarray(inputs[k], dtype=f32)
    i3, i2, jsym = meta

    x3 = g('inputStacks')[i3, 0]
    x2 = g('inputStacks')[i2, 0]
    sv = g('symmetryStacks')[jsym, 0]
    e3 = np.tanh(x3 @ g('box_W') + g('box_b'))
    e2 = np.tanh(x2 @ g('box_W') + g('box_b'))
    ek = np.concatenate([e3.reshape(KJ, 128).T, e2.reshape(KJ, 128).T],
                        axis=1).astype(f16)           # [128, 16]

    adj_Wl, adj_Wr, adj_W2 = g('adj_Wl'), g('adj_Wr'), g('adj_W2')
    sym_Wl, sym_W2, sym_Wr = g('sym_Wl'), g('sym_W2'), g('sym_Wr')
    sym_b1 = g('sym_bl') + g('sym_br')

    def rowpack(W, nchunk):
        return np.ascontiguousarray(
            W.reshape(nchunk, 128, W.shape[1]).transpose(1, 0, 2)
            .reshape(128, nchunk * W.shape[1])).astype(f16)

    awlf = rowpack(adj_Wl, KJ)
    awrf = rowpack(adj_Wr, KJ)
    aw2f = rowpack(adj_W2, HK)
    ablf = np.ascontiguousarray(g('adj_bl')[None, :]).astype(f16)
    b2row = np.ascontiguousarray(g('adj_b2')[None, :]).astype(f16)
    onesf = np.ones((1, 1), f32)
    ones1h = np.ones((1, 1), f16)

    def pack_w1(W, c):
        # [F, H] -> core slice [F, HC] -> [128, KJ*HC]; block (j, m) at
        # cols (j*MC + m)*128, i.e. W-stationary [K=128, M=128] tiles
        s = W[:, c * HC:(c + 1) * HC]
        return np.ascontiguousarray(
            s.reshape(KJ, 128, MC, 128).reshape(KJ, 128, HC)
            .transpose(1, 0, 2).reshape(128, KJ * HC)).astype(f16)

    def pack_w2t(W, c):
        # rows slice [HC, F] -> tiles (k, mz) at col (k*KJ+mz)*128
        s = W[c * HC:(c + 1) * HC, :].reshape(MC, 128, KJ, 128)
        return np.ascontiguousarray(
            s.transpose(1, 0, 2, 3).reshape(128, MC * KJ * 128)).astype(f16)

    in_maps = []
    for c in range(NCORES):
        sv9w = np.zeros((16, 1 + HC), f16)
        sv9w[:SYMD, 0] = sv.astype(f16)
        sv9w[SYMD, 0] = 1.0
        sv9w[:SYMD, 1:] = sym_Wr[:, c * HC:(c + 1) * HC].astype(f16)
        sv9w[SYMD, 1:] = sym_b1[c * HC:(c + 1) * HC].astype(f16)
        in_maps.append({
            "ek": ek, "ablf": ablf, "b2row": b2row, "sv9w": sv9w,
            "onesf": onesf, "ones1h": ones1h,
            "awlf": awlf, "awrf": awrf, "aw2f": aw2f,
            "swl": pack_w1(sym_Wl, c),
            "sw2t": pack_w2t(sym_W2, c),
        })
    return in_maps


# ---------------------------------------------------------------------------
# General fallback: arbitrary sliced DAGs (H-split + AllGather per node)
# ---------------------------------------------------------------------------

def _build_program(nodes, root, box_pos, sym_pos, nb, ns, need_zero):
    import concourse.bacc as bacc
    import concourse.mybir as mybir
    import concourse.tile as tile

    dt = mybir.dt.float32
    dt16 = mybir.dt.float16
    Tanh = mybir.ActivationFunctionType.Tanh
    nc = bacc.Bacc("TRN2", target_bir_lowering=False, debug=False,
                   enable_asserts=False, num_devices=NCORES)

    def din(name, shape, dty):
        return nc.dram_tensor(name, list(shape), dty, kind="ExternalInput")
    d_xz = din("xz", [BOX + 1, nb], dt16)
    d_boxw = din("boxw", [BOX + 1, F], dt16)
    d_awl = din("awl", [128, KJ * HC], dt16)
    d_awr = din("awr", [128, KJ * HC], dt16)
    d_abl = din("abl", [1, HC], dt16)
    d_aw2 = din("aw2", [128, MC * F], dt16)
    d_ab2 = din("ab2", [1, F], dt)
    d_swl = din("swl", [128, KJ * HC], dt16)
    d_swr9 = din("swr9", [SYMD + 1, HC], dt16)
    d_sw2 = din("sw2", [128, MC * F], dt16)
    d_sb2 = din("sb2", [1, F], dt)
    d_sv1 = din("sv1", [SYMD + 1, ns], dt16)
    d_ones = din("ones9", [NCORES + 1, 1], dt)
    d_ones1h = din("ones1h", [1, 1], dt16)
    d_out = nc.dram_tensor("root_t", [128, KJ], dt, kind="ExternalOutput")
    d_pout = nc.dram_tensor("part_out", [1, F], dt, kind="ExternalOutput")
    host_root = root is not None and root[0] == "node"

    n_adj = sum(1 for t, _, _ in nodes if t == 'adj')
    n_sym = len(nodes) - n_adj
    any_exchange = any(
        not (host_root and k == root[1]) for k in range(len(nodes)))
    groups = [list(range(NCORES))]

    with tile.TileContext(nc) as tc:
        with (
            tc.tile_pool(name="wp", bufs=1) as wp,
            tc.tile_pool(name="sp", bufs=2) as sp,
            tc.tile_pool(name="rp", bufs=1) as rp,
            tc.tile_pool(name="pp", bufs=1, space="PSUM") as pp,
            tc.tile_pool(name="dp", bufs=1, space="DRAM") as dp,
        ):
            def load(dram, shape, tag, dty=dt16):
                t = wp.tile(list(shape), dty, tag=tag)
                nc.sync.dma_start(t[:], dram[:])
                return t

            t_ones = load(d_ones, [NCORES + 1, 1], "ones", dt)
            t_ones1h = load(d_ones1h, [1, 1], "ones1h")
            t_boxw = load(d_boxw, [BOX + 1, F], "boxw")
            t_xz = load(d_xz, [BOX + 1, nb], "xz")
            t_awl = t_awr = t_abl = t_aw2 = None
            t_swl = t_swr9 = t_sw2 = t_sv1 = None
            if n_adj:
                t_awl = load(d_awl, [128, KJ * HC], "awl")
                t_awr = load(d_awr, [128, KJ * HC], "awr")
                t_abl = load(d_abl, [1, HC], "abl")
                t_aw2 = load(d_aw2, [128, MC * F], "aw2")
            if n_sym:
                t_swl = load(d_swl, [128, KJ * HC], "swl")
                t_swr9 = load(d_swr9, [SYMD + 1, HC], "swr9")
                t_sw2 = load(d_sw2, [128, MC * F], "sw2")
                t_sv1 = load(d_sv1, [SYMD + 1, ns], "sv1")
            t_zero = None
            if need_zero:
                t_zero = rp.tile([128, KJ], dt, tag="zero")
                nc.gpsimd.memset(t_zero[:], 0.0)

            if any_exchange:
                warm_in = dp.tile([1, 1], dt, tag="warmin")
                warm_out = dp.tile([NCORES, 1], dt, tag="warmout")
                nc.gpsimd.dma_start(warm_in[:], d_ones[0:1, :])
                nc.gpsimd.collective_compute(
                    "AllGather", mybir.AluOpType.bypass,
                    replica_groups=groups,
                    ins=[warm_in[:].opt()], outs=[warm_out[:].opt()])
                nc.gpsimd.dma_start(t_ones[0:1, :], warm_out[0:1, :])

            # --- box encodings, K-major: col m*nb + t = chunk m of box t ---
            ps_box = pp.tile([128, KJ * nb], dt, tag="psbox")
            for m in range(KJ):
                nc.tensor.matmul(ps_box[:, m * nb:(m + 1) * nb],
                                 t_boxw[:, m * 128:(m + 1) * 128],
                                 t_xz[:], start=True, stop=True)
            t_bx = rp.tile([128, KJ * nb], dt16, tag="bx")
            nc.scalar.activation(t_bx[:], ps_box[:], Tanh)

            res_tiles = []

            def col(src, j):
                """K-major chunk j ([128,1] rhs) of a node-input vector."""
                if src is None:
                    return t_zero[:, j:j + 1]
                if src[0] == 'box':
                    t = box_pos[src[1]]
                    return t_bx[:, j * nb + t:j * nb + t + 1]
                return res_tiles[src[1]][:, j:j + 1]

            for k, (typ, a, b) in enumerate(nodes):
                # ---- layer 1: pre[HC] in K-major [128, MC] ----
                ps1 = pp.tile([128, MC], dt, tag="ps1")
                wl = t_awl if typ == 'adj' else t_swl
                for m in range(MC):
                    for j in range(KJ):
                        nc.tensor.matmul(
                            ps1[:, m:m + 1],
                            wl[:, (j * MC + m) * 128:(j * MC + m + 1) * 128],
                            col(a, j), start=(j == 0), stop=False)
                    if typ == 'adj':
                        for j in range(KJ):
                            nc.tensor.matmul(
                                ps1[:, m:m + 1],
                                t_awr[:, (j * MC + m) * 128:(j * MC + m + 1) * 128],
                                col(b, j), start=False, stop=False)
                        nc.tensor.matmul(ps1[:, m:m + 1],
                                         t_abl[:, m * 128:(m + 1) * 128],
                                         t_ones1h[:, :], start=False, stop=True)
                    else:
                        if b is None:
                            # missing sym param == zeros: keep only the bias row
                            nc.tensor.matmul(ps1[:, m:m + 1],
                                             t_swr9[SYMD:SYMD + 1,
                                                    m * 128:(m + 1) * 128],
                                             t_ones1h[:, :],
                                             start=False, stop=True)
                        else:
                            sc = sym_pos[b[1]]
                            nc.tensor.matmul(ps1[:, m:m + 1],
                                             t_swr9[:, m * 128:(m + 1) * 128],
                                             t_sv1[:, sc:sc + 1],
                                             start=False, stop=True)
                th = sp.tile([128, MC], dt16, tag="h1")
                nc.scalar.activation(th[:], ps1[:], Tanh)

                # ---- layer 2: partial [1, F] (row-major, pre-activation) ----
                w2 = t_aw2 if typ == 'adj' else t_sw2
                ps2a = pp.tile([1, 512], dt, tag="ps2a")
                ps2b = pp.tile([1, 512], dt, tag="ps2b")
                for half, pst in ((0, ps2a), (1, ps2b)):
                    for kk in range(MC):
                        nc.tensor.matmul(
                            pst[:, :],
                            th[:, kk:kk + 1],
                            w2[:, kk * F + half * 512: kk * F + half * 512 + 512],
                            start=(kk == 0), stop=(kk == MC - 1))
                t_part = sp.tile([1, F], dt, tag="part")
                nc.vector.tensor_copy(t_part[0:1, 0:512], ps2a[:, :])
                nc.vector.tensor_copy(t_part[0:1, 512:1024], ps2b[:, :])

                if host_root and k == root[1]:
                    # root node: emit per-core partials; host sums+bias+tanh
                    nc.sync.dma_start(d_pout[:], t_part[:])
                    res_tiles.append(None)
                    continue

                # ---- exchange: AllGather partials, reduce + bias + tanh ----
                ccin = dp.tile([1, F], dt, tag=f"ccin{k}")
                ccout = dp.tile([NCORES, F], dt, tag=f"ccout{k}")
                nc.sync.dma_start(ccin[:], t_part[:])
                nc.gpsimd.collective_compute(
                    "AllGather", mybir.AluOpType.bypass,
                    replica_groups=groups,
                    ins=[ccin[:].opt()], outs=[ccout[:].opt()])
                t_P = sp.tile([NCORES + 1, F], dt, tag="P")
                nc.sync.dma_start(t_P[0:NCORES, :], ccout[:])
                nc.sync.dma_start(t_P[NCORES:NCORES + 1, :],
                                  (d_ab2 if typ == 'adj' else d_sb2)[:])
                psr = pp.tile([128, KJ], dt, tag="psr")
                for m in range(KJ):
                    nc.tensor.matmul(psr[:, m:m + 1],
                                     t_P[:, m * 128:(m + 1) * 128],
                                     t_ones[:, :], start=True, stop=True)
                t_res = rp.tile([128, KJ], dt16, tag=f"res{k}")
                nc.scalar.activation(t_res[:], psr[:], Tanh)
                res_tiles.append(t_res)

            # ---- root -> output ----
            if root is None:
                nc.sync.dma_start(d_out[:], t_zero[:])
            elif root[0] == 'node':
                pass  # root node handled above via part_out
            else:  # box leaf
                t_stage = rp.tile([128, KJ], dt, tag="rootstage")
                t = box_pos[root[1]]
                for j in range(KJ):
                    nc.vector.tensor_copy(t_stage[:, j:j + 1],
                                          t_bx[:, j * nb + t:j * nb + t + 1])
                nc.sync.dma_start(d_out[:], t_stage[:])

    nc.compile()
    return nc


def _pack_inputs(inputs, boxes, syms, nb, ns):
    f32, f16 = np.float32, np.float16
    g = lambda k: np.asarray(inputs[k], dtype=f32)
    inputStacks, symmetryStacks = g('inputStacks'), g('symmetryStacks')

    xz = np.zeros((BOX + 1, nb), f16)
    for t, i in enumerate(boxes):
        xz[:BOX, t] = inputStacks[i, 0].astype(f16)
        xz[BOX, t] = 1.0
    boxw = np.ascontiguousarray(
        np.concatenate([g('box_W'), g('box_b')[None, :]], axis=0)).astype(f16)
    sv1 = np.zeros((SYMD + 1, ns), f16)
    for t, j in enumerate(syms):
        sv1[:SYMD, t] = symmetryStacks[j, 0].astype(f16)
        sv1[SYMD, t] = 1.0
    ones9 = np.ones((NCORES + 1, 1), f32)
    ones1h = np.ones((1, 1), f16)
    ab2 = np.ascontiguousarray(g('adj_b2')[None, :])
    sb2 = np.ascontiguousarray(g('sym_b2')[None, :])

    def pack_w1(W, c):
        s = W[:, c * HC:(c + 1) * HC]
        return np.ascontiguousarray(
            s.reshape(KJ, 128, HC).transpose(1, 0, 2).reshape(
                128, KJ * HC)).astype(f16)

    def pack_w2(W, c):
        s = W[c * HC:(c + 1) * HC, :]
        return np.ascontiguousarray(
            s.reshape(MC, 128, F).transpose(1, 0, 2).reshape(
                128, MC * F)).astype(f16)

    adj_Wl, adj_Wr, adj_W2 = g('adj_Wl'), g('adj_Wr'), g('adj_W2')
    sym_Wl, sym_W2, sym_Wr = g('sym_Wl'), g('sym_W2'), g('sym_Wr')
    sym_b1 = g('sym_bl') + g('sym_br')
    adj_bl = g('adj_bl')

    in_maps = []
    for c in range(NCORES):
        swr9 = np.ascontiguousarray(np.concatenate(
            [sym_Wr[:, c * HC:(c + 1) * HC],
             sym_b1[None, c * HC:(c + 1) * HC]], axis=0)).astype(f16)
        in_maps.append({
            "xz": xz, "boxw": boxw, "sv1": sv1,
            "ones9": ones9, "ones1h": ones1h, "ab2": ab2, "sb2": sb2,
            "awl": pack_w1(adj_Wl, c), "awr": pack_w1(adj_Wr, c),
            "abl": np.ascontiguousarray(
                adj_bl[None, c * HC:(c + 1) * HC]).astype(f16),
            "aw2": pack_w2(adj_W2, c),
            "swl": pack_w1(sym_Wl, c), "swr9": swr9,
            "sw2": pack_w2(sym_W2, c),
        })
    return in_maps


# ---------------------------------------------------------------------------
# Entry point
# ---------------------------------------------------------------------------

def build_for_inputs(inputs):
    """Build (or fetch cached) compiled program + packed inputs."""
    ops = np.asarray(inputs['operations'])
    ops0 = ops[:, 0].astype(np.int64)
    nodes, root = _build_slice(ops0)
    boxes, syms, need_zero = _collect_leaves(nodes, root)
    nb, ns = max(1, len(boxes)), max(1, len(syms))

    use_fast = _canonical(nodes, root)
    key = repr((nodes, root, nb, ns, need_zero, use_fast))
    if key not in _CACHE:
        if use_fast:
            _CACHE[key] = _build_program_fast()
        else:
            box_pos = {b: i for i, b in enumerate(boxes)}
            sym_pos = {s: i for i, s in enumerate(syms)}
            _CACHE[key] = _build_program(nodes, root, box_pos, sym_pos,
                                         nb, ns, need_zero)
    nc = _CACHE[key]
    if use_fast:
        meta = (nodes[0][1][1], nodes[0][2][1], nodes[1][2][1])
        in_maps = _pack_inputs_fast(inputs, meta)
    else:
        box_pos = {b: i for i, b in enumerate(boxes)}
        sym_pos = {s: i for i, s in enumerate(syms)}
        in_maps = _pack_inputs(inputs, boxes, syms, nb, ns)
    return nc, in_maps, (nodes, root, use_fast)


def assemble_output(results, nodes, root, use_fast, inputs):
    """Host-side unshard: combine per-core outputs into the root vector."""
    if use_fast:
        parts = np.zeros((128, KJ), np.float64)
        for c in range(NCORES):
            parts += np.asarray(results[c]["pout_k"], np.float64)
        b2 = np.asarray(inputs['sym_b2'], np.float64).reshape(KJ, 128).T
        return np.tanh(parts + b2).astype(np.float32).T.ravel()
    if root is not None and root[0] == 'node':
        parts = np.stack([np.asarray(results[c]["part_out"], np.float32)[0]
                          for c in range(NCORES)])
        b2 = np.asarray(
            inputs['adj_b2' if nodes[root[1]][0] == 'adj' else 'sym_b2'],
            np.float32)
        return np.tanh(parts.sum(axis=0) + b2).astype(np.float32)
    root_t = np.asarray(results[0]["root_t"], np.float32)
    return np.ascontiguousarray(root_t.T.ravel())


def kernel(**inputs) -> np.ndarray:
    from concourse.bass_utils import run_bass_kernel_spmd

    nc, in_maps, (nodes, root, use_fast) = build_for_inputs(inputs)
    res = run_bass_kernel_spmd(nc, in_maps, core_ids=list(range(NCORES)))
    return assemble_output(res.results, nodes, root, use_fast, inputs)
